# revision 32
# baseline (speedup 1.0000x reference)
"""MoE layer (B=4,S=2048,D=1024,I=4096,E=8,top_k=2) on 8 TRN2 NeuronCores.

Strategy: expert-parallel sparse dispatch.
 - Host: router matmul (tiny), top-k + softmax gates, gather tokens per expert.
 - Device (core e == expert e): yT = (gelu(x @ W1) @ W2 + b2) * gate, with
   x/W in bf16 on the TensorEngine, fp32 PSUM accumulation, token dim padded
   to a multiple of 128 and processed in 512-wide chunks.
 - Host: scatter-add the K=2 gated expert outputs back to [B,S,D].
"""

import os

import ml_dtypes
import numpy as np

import concourse.bass as bass
import concourse.bacc as bacc
import concourse.mybir as mybir
import concourse.tile as tile
from concourse.bass_utils import run_bass_kernel_spmd

BF16 = mybir.dt.bfloat16
F32 = mybir.dt.float32
P = 128
N_CORES = 8

# Filled with the profiled exec time (ns) of the last run when
# BASS_KERNEL_TRACE=1 is set in the environment (used by test.py).
LAST_EXEC_NS = None
LAST_RESULTS = None

_cache: dict = {}


def _chunks_for(C: int) -> list[int]:
    """[512, ..., remainder]. Measured faster than equal-width chunks:
    N=512 matmuls hit the 216ns streaming bound and the N=128 tail is
    cheap, while e.g. N=448 matmuls miss the N/2.4+2.5ns model."""
    chunks = [512] * (C // 512)
    if C % 512:
        chunks.append(C % 512)
    return chunks


def _build(C: int, D: int, I: int):
    """Per-core FFN program: one expert, C token slots (multiple of 128)."""
    KD = D // P  # k-tiles for contraction over D
    KI = I // P  # k-tiles for contraction over I
    ND = D // P  # output row tiles

    nc = bacc.Bacc()
    xT = nc.declare_dram_parameter("xT", [D, C], BF16, isOutput=False)
    w1 = nc.declare_dram_parameter("w1", [D, I], BF16, isOutput=False)
    b1 = nc.declare_dram_parameter("b1", [P, I // P], F32, isOutput=False)
    w2 = nc.declare_dram_parameter("w2", [I, D], BF16, isOutput=False)
    b2 = nc.declare_dram_parameter("b2", [P, D // P], F32, isOutput=False)
    g = nc.declare_dram_parameter("g", [P, C], F32, isOutput=False)
    yT = nc.declare_dram_parameter("yT", [D, C], F32, isOutput=True)

    xTr = xT[:].rearrange("(k p) c -> k p c", p=P)
    w1r = w1[:].rearrange("(k p) i -> k p i", p=P)
    w2r = w2[:].rearrange("(k p) d -> k p d", p=P)
    yTr = yT[:].rearrange("(k p) c -> k p c", p=P)

    with tile.TileContext(nc) as tc:
        with (
            tc.tile_pool(name="wpool", bufs=1) as wpool,
            tc.tile_pool(name="cpool", bufs=1) as cpool,
            tc.tile_pool(name="xpool", bufs=2) as xpool,
            tc.tile_pool(name="hpool", bufs=1) as hpool,
            tc.tile_pool(name="ypool", bufs=4) as ypool,
            tc.tile_pool(name="pspool", bufs=7, space="PSUM") as pspool,
        ):
            chunks = _chunks_for(C)
            # First-chunk x tiles are queued before the bulk of w1 so the
            # first m1 groups are not stuck behind 8MB of weight DMA.
            b1_sb = cpool.tile([P, I // P], F32, tag="b1")
            nc.sync.dma_start(out=b1_sb[:], in_=b1[:])
            x_first = []
            for k in range(KD):
                t = xpool.tile([P, chunks[0]], BF16, tag=f"x_{k}")
                nc.sync.dma_start(out=t[:], in_=xTr[k][:, : chunks[0]])
                x_first.append(t)
            # Resident W1 (bf16, 64KB/part) in 4 column-chunks per k-tile,
            # loaded column-chunk-major: m1 group i only needs chunk i//8,
            # so PE can start after ~3MB instead of the full 8MB.
            JW = 4
            JCOL = I // JW
            w1_sb = [[None] * JW for _ in range(KD)]
            for j in range(JW):
                for k in range(KD):
                    t = wpool.tile([P, JCOL], BF16, tag=f"w1_{k}_{j}")
                    nc.sync.dma_start(
                        out=t[:], in_=w1r[k][:, j * JCOL : (j + 1) * JCOL]
                    )
                    w1_sb[k][j] = t
            # The Activation encoding fits a single sync wait. Every gelu's
            # PSUM RAW wait (PE sem) dominates its h-slot WAR tick, so the
            # only extra wait a gelu could need is the b1 DMA — absorb it
            # once with a 1-element warm-up copy so ACT's vector clock has
            # observed that DMA before the first real gelu.
            warm = cpool.tile([1, 1], F32, tag="warm")
            warm2 = cpool.tile([1, 1], F32, tag="warm2")
            nc.scalar.copy(warm[:], b1_sb[:1, :1])

            # HAM warm-up: ~7us of dummy matmuls on zeroed scratch while the
            # first w1/x DMAs stream, so real matmuls start at 2.4 GHz
            # instead of paying the 1.2 GHz cold window. (Best measured
            # config: 16 dummies, GpSimd memset — DVE memset and 32 dummies
            # both measured slightly worse; startup is DMA-bandwidth-bound.)
            scratch = cpool.tile([P, 512], BF16, tag="scratch")
            nc.gpsimd.memset(scratch[:], 0.0)
            for _ in range(2):
                pw = pspool.tile([P, 512], F32, tag="ps")
                for k in range(KD):
                    nc.tensor.matmul(
                        pw[:],
                        scratch[:, :P],
                        scratch[:],
                        start=(k == 0),
                        stop=(k == KD - 1),
                    )

            # W2/b2/g are not needed until the first m2 phase — their DMAs
            # are emitted after chunk-0 m1 below so they don't queue ahead of
            # the chunk-0 x tiles in the DMA FIFOs (measured 55 us PE stall).
            w2_sb = []
            b2_dve = None
            g_dve = None

            def _load_phase2():
                for k in range(KI):
                    t = wpool.tile([P, D], BF16, tag=f"w2_{k}")
                    nc.sync.dma_start(out=t[:], in_=w2r[k])
                    w2_sb.append(t)
                b2_sb = cpool.tile([P, D // P], F32, tag="b2")
                nc.sync.dma_start(out=b2_sb[:], in_=b2[:])
                g_sb = cpool.tile([P, C], F32, tag="g")
                nc.sync.dma_start(out=g_sb[:], in_=g[:])
                b2_stage = cpool.tile([P, D // P], F32, tag="b2v")
                nc.vector.tensor_copy(b2_stage[:], b2_sb[:])
                g_stage = cpool.tile([P, C], F32, tag="gv")
                nc.vector.tensor_copy(g_stage[:], g_sb[:])
                return b2_stage, g_stage

            c0 = 0
            prev_h_last = None
            for ci, cw in enumerate(chunks):
                if ci == 0:
                    x_sb = x_first
                else:
                    x_sb = []
                    for k in range(KD):
                        t = xpool.tile([P, cw], BF16, tag=f"x_{k}")
                        nc.sync.dma_start(out=t[:], in_=xTr[k][:, c0 : c0 + cw])
                        x_sb.append(t)
                # hT = gelu(x @ W1 + b1), tiled [128 of I, cw]
                if prev_h_last is not None:
                    # Advance ACT's observed self-tick past ALL of the
                    # previous chunk's gelus so the h-tile WAW deps below
                    # don't each need their own (second) sync wait.
                    nc.scalar.copy(warm[:], prev_h_last[:1, :1])
                    nc.scalar.copy(warm2[:], warm[:])
                h_sb = []
                for i in range(KI):
                    ps = pspool.tile([P, cw], F32, tag="ps")
                    jw, jo = divmod(i * P, JCOL)
                    for k in range(KD):
                        nc.tensor.matmul(
                            ps[:],
                            w1_sb[k][jw][:, jo : jo + P],
                            x_sb[k][:],
                            start=(k == 0),
                            stop=(k == KD - 1),
                        )
                    ht = hpool.tile([P, cw], BF16, tag=f"h_{i}")
                    nc.scalar.activation(
                        ht[:],
                        ps[:],
                        mybir.ActivationFunctionType.Gelu,
                        bias=b1_sb[:, i : i + 1],
                    )
                    h_sb.append(ht)
                prev_h_last = h_sb[-1]
                if g_dve is None:
                    b2_dve, g_dve = _load_phase2()
                # yT = (hT' @ W2 + b2) * g, tiled [128 of D, cw]
                for d in range(ND):
                    ps = pspool.tile([P, cw], F32, tag="ps")
                    for k in range(KI):
                        nc.tensor.matmul(
                            ps[:],
                            w2_sb[k][:, d * P : (d + 1) * P],
                            h_sb[k][:],
                            start=(k == 0),
                            stop=(k == KI - 1),
                        )
                    yt = ypool.tile([P, cw], F32, tag="y")
                    nc.vector.scalar_tensor_tensor(
                        out=yt[:],
                        in0=ps[:],
                        scalar=b2_dve[:, d : d + 1],
                        in1=g_dve[:, c0 : c0 + cw],
                        op0=mybir.AluOpType.add,
                        op1=mybir.AluOpType.mult,
                    )
                    nc.sync.dma_start(out=yTr[d][:, c0 : c0 + cw], in_=yt[:])
                c0 += cw
    nc.compile()
    return nc


def kernel(**inputs) -> np.ndarray:
    global LAST_EXEC_NS, LAST_RESULTS
    x = np.asarray(inputs["x"], dtype=np.float32)
    Wr = np.asarray(inputs["Wr"], dtype=np.float32)
    br = np.asarray(inputs["br"], dtype=np.float32)
    W1 = np.asarray(inputs["W1"], dtype=np.float32)
    b1 = np.asarray(inputs["b1"], dtype=np.float32)
    W2 = np.asarray(inputs["W2"], dtype=np.float32)
    b2 = np.asarray(inputs["b2"], dtype=np.float32)
    K = int(np.asarray(inputs["top_k"]))

    B, S, D = x.shape
    E = Wr.shape[0]
    I = W1.shape[2]
    T = B * S
    xf = x.reshape(T, D)

    # Router (tiny) on host: logits -> top-k (desc, ties -> lower index,
    # matching jax.lax.top_k) -> softmax over the selected k.
    logits = xf @ Wr.T + br
    order = np.argsort(-logits, axis=-1, kind="stable")[:, :K]
    topv = np.take_along_axis(logits, order, axis=-1)
    exv = np.exp(topv - topv.max(axis=-1, keepdims=True))
    gates = (exv / exv.sum(axis=-1, keepdims=True)).astype(np.float32)

    toks, gvals = [], []
    for e in range(E):
        sel = order == e
        tok = np.nonzero(sel.any(axis=-1))[0]
        kidx = np.argmax(sel[tok], axis=-1)
        toks.append(tok)
        gvals.append(gates[tok, kidx].astype(np.float32))

    maxc = max(max(len(t) for t in toks), P)
    C = ((maxc + P - 1) // P) * P

    key = (C, D, I)
    if key not in _cache:
        _cache[key] = _build(C, D, I)
    nc = _cache[key]

    bf = ml_dtypes.bfloat16
    in_maps = []
    for e in range(E):
        n = len(toks[e])
        xTe = np.zeros((D, C), dtype=bf)
        if n:
            xTe[:, :n] = xf[toks[e]].T.astype(bf)
        ge = np.zeros((P, C), dtype=np.float32)
        if n:
            ge[:, :n] = gvals[e][None, :]
        in_maps.append(
            {
                "xT": xTe,
                "w1": np.ascontiguousarray(W1[e].astype(bf)),
                "b1": np.ascontiguousarray(b1[e].reshape(I // P, P).T),
                "w2": np.ascontiguousarray(W2[e].astype(bf)),
                "b2": np.ascontiguousarray(b2[e].reshape(D // P, P).T),
                "g": ge,
            }
        )

    trace = bool(int(os.environ.get("BASS_KERNEL_TRACE", "0")))
    if trace:
        try:
            from antenv.axon_hooks import get_axon_ntff_profile_hook  # noqa: F401
        except ImportError:
            trace = False
    res = run_bass_kernel_spmd(
        nc, in_maps, core_ids=list(range(N_CORES)), trace=trace
    )
    LAST_EXEC_NS = res.exec_time_ns
    LAST_RESULTS = res

    out = np.zeros((T, D), dtype=np.float32)
    for e in range(E):
        n = len(toks[e])
        if n:
            out[toks[e]] += res.results[e]["yT"][:, :n].T
    return out.reshape(B, S, D)
